# revision 1
# baseline (speedup 1.0000x reference)
"""Point-transformer pairwise vector-attention kernel for 8 Trainium2 cores.

Sharding: data-parallel over (batch, query-quarter): core c handles batch
c//4, queries (c%4)*192 : +192, against all 768 keys. No collectives.

Device computes only the O(N^2) pairwise part; all N-sized projections and
the final output projection/residual are exact host-side numpy.
"""

import sys

for _p in ("/opt/trn_rl_repo", "/root/.axon_site/_ro/trn_rl_repo"):
    if _p not in sys.path:
        sys.path.insert(0, _p)

import numpy as np

B, N, C = 2, 768, 64
PH, AH = 32, 32
NQ = 192          # queries per core
I = 4             # queries per i-tile
NIT = NQ // I     # 48 i-tiles
JC = 256          # key-span per psum chunk
NJC = N // JC     # 3 chunks per i-tile
MASK_NEG = 60.0   # logit penalty for masked keys (exp(-60) ~ 9e-27)

_CACHE = {}


def _build_program():
    import concourse.tile as tile
    from concourse import bacc, mybir

    f32 = mybir.dt.float32
    nc = bacc.Bacc("TRN2", target_bir_lowering=False, debug=False, num_devices=8)

    d_pqT = nc.dram_tensor("pqT", [PH, NQ], f32, kind="ExternalInput")
    d_pkT = nc.dram_tensor("pkT", [PH, N], f32, kind="ExternalInput")
    d_qaT = nc.dram_tensor("qaT", [AH, NQ], f32, kind="ExternalInput")
    d_katil = nc.dram_tensor("katil", [AH, I * N], f32, kind="ExternalInput")
    d_vptil = nc.dram_tensor("vptil", [C, I * N], f32, kind="ExternalInput")
    d_onemrow = nc.dram_tensor("onemrow", [2, I * N], f32, kind="ExternalInput")
    d_W2 = nc.dram_tensor("W2", [128, 96], f32, kind="ExternalInput")
    d_W3 = nc.dram_tensor("W3", [AH + 2, C], f32, kind="ExternalInput")
    d_out = nc.dram_tensor("out", [128, NQ], f32, kind="ExternalOutput")

    FD = I * N  # 3072 free elems per i-tile

    with tile.TileContext(nc) as tc:
        with (
            tc.tile_pool(name="const", bufs=1) as cpool,
            tc.tile_pool(name="psum2", bufs=2, space="PSUM") as p2pool,
            tc.tile_pool(name="psum3", bufs=2, space="PSUM") as p3pool,
        ):
            W2sb = cpool.tile([128, 96], f32)
            W3sb = cpool.tile([AH + 2, C], f32)
            pqsb = cpool.tile([PH, NQ], f32)
            pksb = cpool.tile([PH, N], f32)
            qasb = cpool.tile([AH, NQ], f32)
            outsb = cpool.tile([128, NQ], f32)
            nc.sync.dma_start(out=W2sb[:], in_=d_W2[:])
            nc.sync.dma_start(out=W3sb[:], in_=d_W3[:])
            nc.sync.dma_start(out=pqsb[:], in_=d_pqT[:])
            nc.sync.dma_start(out=pksb[:], in_=d_pkT[:])
            nc.sync.dma_start(out=qasb[:], in_=d_qaT[:])

            # persistent ping-pong tiles; broadcast rows prefilled once
            rhs2s, rhs3s, eps = [], [], []
            for k in range(2):
                r2 = cpool.tile([128, FD], f32, tag=f"rhs2_{k}")
                r3 = cpool.tile([AH + 2, FD], f32, tag=f"rhs3_{k}")
                ep = cpool.tile([128, FD], f32, tag=f"ep_{k}")
                nc.sync.dma_start(out=r2[PH : PH + AH, :], in_=d_katil[:])
                nc.sync.dma_start(out=r2[64:128, :], in_=d_vptil[:])
                nc.sync.dma_start(out=r3[AH : AH + 2, :], in_=d_onemrow[:])
                rhs2s.append(r2)
                rhs3s.append(r3)
                eps.append(ep)

            for it in range(NIT):
                i0 = it * I
                rhs2 = rhs2s[it % 2]
                rhs3 = rhs3s[it % 2]
                ep = eps[it % 2]

                # h1 = relu(pq_i - pk_j) into rhs2 rows 0:32
                h1v = rhs2[0:PH, :].rearrange("p (i j) -> p i j", j=N)
                nc.vector.tensor_tensor(
                    out=h1v,
                    in0=pqsb[:, i0 : i0 + I].unsqueeze(2).broadcast_to((PH, I, N)),
                    in1=pksb[:].unsqueeze(1).broadcast_to((PH, I, N)),
                    op=mybir.AluOpType.subtract,
                )
                nc.vector.tensor_scalar_max(rhs2[0:PH, :], rhs2[0:PH, :], 0.0)

                r2v = rhs2[:].rearrange("p (i j) -> p i j", j=N)
                r3v = rhs3[:].rearrange("p (i j) -> p i j", j=N)
                epv = ep[:].rearrange("p (i j) -> p i j", j=N)

                for jc in range(NJC):
                    j0 = jc * JC
                    p2 = p2pool.tile([96, I * JC], f32)
                    p3 = p3pool.tile([C, I * JC], f32)
                    # mm2: [h1; ka; v'] @ W2 -> [vp | h2pre]; 512 cols/matmul
                    for h in range(2):
                        nc.tensor.matmul(
                            p2[:, h * 512 : (h + 1) * 512],
                            lhsT=W2sb[:],
                            rhs=r2v[:, 2 * h : 2 * h + 2, j0 : j0 + JC],
                        )
                    p2v = p2[:].rearrange("p (i j) -> p i j", j=JC)
                    # h2 = relu(h2pre + qa_i) into rhs3 rows 0:32
                    h2slice = r3v[0:AH, :, j0 : j0 + JC]
                    nc.vector.tensor_tensor(
                        out=h2slice,
                        in0=p2v[64:96, :, :],
                        in1=qasb[:, i0 : i0 + I].unsqueeze(2).broadcast_to((AH, I, JC)),
                        op=mybir.AluOpType.add,
                    )
                    nc.vector.tensor_scalar_max(h2slice, h2slice, 0.0)
                    # mm3: [h2; 1; 1-mask] @ [am_w2; am_b2; -60] -> logits
                    for h in range(2):
                        nc.tensor.matmul(
                            p3[:, h * 512 : (h + 1) * 512],
                            lhsT=W3sb[:],
                            rhs=r3v[:, 2 * h : 2 * h + 2, j0 : j0 + JC],
                        )
                    p3v = p3[:].rearrange("p (i j) -> p i j", j=JC)
                    # e = exp(logits) ; prod = e * vp
                    nc.scalar.activation(
                        out=epv[0:C, :, j0 : j0 + JC],
                        in_=p3v,
                        func=mybir.ActivationFunctionType.Exp,
                    )
                    nc.vector.tensor_tensor(
                        out=epv[64:128, :, j0 : j0 + JC],
                        in0=epv[0:C, :, j0 : j0 + JC],
                        in1=p2v[0:C, :, :],
                        op=mybir.AluOpType.mult,
                    )

                # den (rows 0:64) and num (rows 64:128) in one grouped reduce
                nc.vector.tensor_reduce(
                    out=outsb[:, i0 : i0 + I],
                    in_=epv,
                    axis=mybir.AxisListType.X,
                    op=mybir.AluOpType.add,
                )

            nc.sync.dma_start(out=d_out[:], in_=outsb[:])

    nc.compile()
    return nc


def _prepare_inputs(inputs):
    coords = np.asarray(inputs["coords"], np.float32)
    mask = np.asarray(inputs["mask"]).astype(np.float32)
    g = lambda name: np.asarray(inputs[name], np.float32)
    ce_w1, ce_b1, ce_w2, ce_b2 = g("ce_w1"), g("ce_b1"), g("ce_w2"), g("ce_b2")
    wq, wk, wv = g("wq"), g("wk"), g("wv")
    pm_w1, pm_b1, pm_w2, pm_b2 = g("pm_w1"), g("pm_b1"), g("pm_w2"), g("pm_b2")
    am_w1, am_b1, am_w2, am_b2 = g("am_w1"), g("am_b1"), g("am_w2"), g("am_b2")

    x = np.maximum(coords @ ce_w1 + ce_b1, 0.0) @ ce_w2 + ce_b2  # [B,N,C]
    q = x @ wq
    k = x @ wk
    v = x @ wv
    pq = coords @ pm_w1 + pm_b1           # [B,N,PH]
    pk = coords @ pm_w1
    cb = pm_b2 @ am_w1 + am_b1            # [AH]
    qa = q @ am_w1 + cb                   # [B,N,AH]
    ka = k @ am_w1
    vp = v + pm_b2                        # [B,N,C]

    W2 = np.zeros((128, 96), np.float32)
    W2[0:PH, 0:C] = pm_w2
    W2[64:128, 0:C] = np.eye(C, dtype=np.float32)
    W2[0:PH, C : C + AH] = pm_w2 @ am_w1
    W2[PH : PH + AH, C : C + AH] = -np.eye(AH, dtype=np.float32)
    W3 = np.concatenate(
        [am_w2, am_b2[None, :], -MASK_NEG * np.ones((1, C), np.float32)], axis=0
    ).astype(np.float32)

    in_maps = []
    for c in range(8):
        b = c // 4
        qo = (c % 4) * NQ
        in_maps.append(
            {
                "pqT": np.ascontiguousarray(pq[b, qo : qo + NQ].T),
                "pkT": np.ascontiguousarray(pk[b].T),
                "qaT": np.ascontiguousarray(qa[b, qo : qo + NQ].T),
                "katil": np.ascontiguousarray(np.tile(ka[b].T, (1, I))),
                "vptil": np.ascontiguousarray(np.tile(vp[b].T, (1, I))),
                "onemrow": np.ascontiguousarray(
                    np.stack(
                        [
                            np.ones(I * N, np.float32),
                            np.tile(1.0 - mask[b], I),
                        ]
                    )
                ),
                "W2": W2,
                "W3": W3,
            }
        )
    return in_maps, x, g("out_w"), g("out_b")


def _run(inputs, trace=False):
    from concourse.bass_utils import run_bass_kernel_spmd

    if "nc" not in _CACHE:
        _CACHE["nc"] = _build_program()
    nc = _CACHE["nc"]

    in_maps, x, out_w, out_b = _prepare_inputs(inputs)
    if trace:
        _install_ntff_hook()
    r = run_bass_kernel_spmd(nc, in_maps, core_ids=list(range(8)), trace=trace)

    y = np.zeros((B, N, C), np.float32)
    for c in range(8):
        b = c // 4
        qo = (c % 4) * NQ
        o = r.results[c]["out"]  # [128, NQ]
        den = o[0:C, :].T        # [NQ, C]
        num = o[C:128, :].T
        y[b, qo : qo + NQ] = num / den
    y = y @ out_w + out_b + x
    return y.astype(np.float32), r


def kernel(**inputs):
    y, _ = _run(inputs, trace=False)
    return y


def _install_ntff_hook():
    """The image lacks antenv.axon_hooks; provide the NTFF profile hook so
    run_bass_kernel_spmd(trace=True) can capture HW timing via libaxon."""
    import contextlib, ctypes, types

    try:
        from antenv import axon_hooks  # noqa: F401
        return
    except ImportError:
        pass

    so_path = "/opt/axon/libaxon_pjrt.so"
    lib = ctypes.CDLL(so_path)
    if not hasattr(lib, "axon_start_nrt_profile"):
        return
    lib.axon_start_nrt_profile.argtypes = [
        ctypes.POINTER(ctypes.c_int64),
        ctypes.c_size_t,
    ]
    lib.axon_start_nrt_profile.restype = ctypes.c_int64
    lib.axon_stop_nrt_profile.argtypes = [ctypes.c_char_p]
    lib.axon_stop_nrt_profile.restype = ctypes.c_int64

    @contextlib.contextmanager
    def _hook(output_dir, device_ids):
        import jax

        jax.devices()
        if device_ids:
            ids = (ctypes.c_int64 * len(device_ids))(*device_ids)
            rc = lib.axon_start_nrt_profile(ids, len(device_ids))
        else:
            rc = lib.axon_start_nrt_profile(None, 0)
        if rc != 0:
            raise RuntimeError(f"axon_start_nrt_profile rc={rc}")
        try:
            yield
        finally:
            n = lib.axon_stop_nrt_profile(str(output_dir).encode())
            print(f"profile: {n} file(s) written to {output_dir}")

    mod = types.ModuleType("antenv.axon_hooks")
    mod.get_axon_ntff_profile_hook = lambda: _hook
    mod.set_axon_ntff_profile_hook = lambda h: None
    sys.modules["antenv.axon_hooks"] = mod
    import antenv

    antenv.axon_hooks = mod
